# revision 1
# baseline (speedup 1.0000x reference)
"""DeepSeek-style sparse attention (causal + local-window softmax mix) on 8 trn2 cores.

Sharding: tensor-parallel over heads. 16 Q heads / 4 KV heads; core c gets
Q heads {2c, 2c+1} and their shared KV head c//2. Each core computes a
partial output projection (its 256 rows of wo); the host sums the 8 partials.

All matmuls run as float32r (full-rate fp32 mode on the PE array).
Attention is computed in transposed layout S^T[k, q] so that softmax sums are
PE ones-matmuls and PV needs no transposes of the probability tiles.
Softmax skips the max-subtraction pass: scores are O(10), exp cannot overflow,
and masked entries use an additive -1e30 (exp underflows to exactly 0).
The causal and local softmaxes share one set of exp(S) tiles: with a shared
(omitted) max, local probabilities are causal ones restricted to the window.
"""

import math

import numpy as np

import concourse.bass as bass
import concourse.mybir as mybir
import concourse.tile as tile
from concourse import bacc
from concourse.bass_utils import run_bass_kernel_spmd

P = 128
T = 2048
C = 2048
D = 128                 # head dim
N_HEAD = 16
N_KV_HEAD = 4
H_LOC = 2               # q heads per core
N_CORES = 8
TB = T // P             # 16 t blocks of 128
KC = C // P             # 16 contraction chunks of 128
QT = T // 512           # 4 t quarters (projection phase)
G = T // 256            # 8 query groups of 256 (attention phase)
NEG = -1.0e30
F32 = mybir.dt.float32
F32R = mybir.dt.float32r

_PROGRAM_CACHE = {}


def _host_constants():
    """Host-precomputed constant tensors shipped as kernel inputs."""
    i = np.arange(P)
    tril = (i[:, None] <= i[None, :])          # [k_loc, q_loc]: valid iff k <= q
    mc_add = np.where(tril, 0.0, NEG).astype(np.float32)
    mc_mul = tril.astype(np.float32)
    ma_add = np.concatenate([mc_add, np.zeros((P, P), np.float32)], axis=1)
    mb_add = np.concatenate([np.full((P, P), NEG, np.float32), mc_add], axis=1)
    ma_mul = np.concatenate([mc_mul, np.ones((P, P), np.float32)], axis=1)
    mb_mul = np.concatenate([np.zeros((P, P), np.float32), mc_mul], axis=1)

    inv_freq = 1.0 / (10000.0 ** (np.arange(0, D, 2, dtype=np.float64) / D))
    t = np.arange(T, dtype=np.float64)
    freqs = t[:, None] * inv_freq[None, :]              # [T, D/2]
    emb = np.concatenate([freqs, freqs], axis=-1)       # [T, D]
    cos_t = np.cos(emb).T.astype(np.float32).copy()     # [D, T]
    sin_t = np.sin(emb).T.astype(np.float32).copy()

    # rot matmul weights: lhsT[d, d'] with out[d'] = -q[d'+64] (d'<64), q[d'-64] (d'>=64)
    rot_t = np.zeros((P, P), np.float32)
    rot_t[64 + np.arange(64), np.arange(64)] = -1.0
    rot_t[np.arange(64), 64 + np.arange(64)] = 1.0

    return {
        "ma_add": ma_add, "mb_add": mb_add,
        "ma_mul": ma_mul, "mb_mul": mb_mul,
        "cos_t": cos_t, "sin_t": sin_t, "rot_t": rot_t,
        "ident": np.eye(P, dtype=np.float32),
        "ones_c": np.ones((P, 1), np.float32),
    }


def _emit(nc):
    x = nc.dram_tensor("x", [T, C], F32, kind="ExternalInput").ap()
    wq = nc.dram_tensor("wq", [C, H_LOC * D], F32, kind="ExternalInput").ap()
    wk = nc.dram_tensor("wk", [C, D], F32, kind="ExternalInput").ap()
    wv = nc.dram_tensor("wv", [C, D], F32, kind="ExternalInput").ap()
    wo = nc.dram_tensor("wo", [H_LOC * D, C], F32, kind="ExternalInput").ap()
    cos_t = nc.dram_tensor("cos_t", [P, T], F32, kind="ExternalInput").ap()
    sin_t = nc.dram_tensor("sin_t", [P, T], F32, kind="ExternalInput").ap()
    rot_t = nc.dram_tensor("rot_t", [P, P], F32, kind="ExternalInput").ap()
    ident_d = nc.dram_tensor("ident", [P, P], F32, kind="ExternalInput").ap()
    ones_d = nc.dram_tensor("ones_c", [P, 1], F32, kind="ExternalInput").ap()
    ma_add = nc.dram_tensor("ma_add", [P, 256], F32, kind="ExternalInput").ap()
    mb_add = nc.dram_tensor("mb_add", [P, 256], F32, kind="ExternalInput").ap()
    ma_mul = nc.dram_tensor("ma_mul", [P, 256], F32, kind="ExternalInput").ap()
    mb_mul = nc.dram_tensor("mb_mul", [P, 256], F32, kind="ExternalInput").ap()
    out = nc.dram_tensor("out", [T, C], F32, kind="ExternalOutput").ap()

    with tile.TileContext(nc) as tc:
        from contextlib import ExitStack
        with ExitStack() as ctx:
            consts = ctx.enter_context(tc.tile_pool(name="consts", bufs=1))
            ident = consts.tile([P, P], F32R)
            nc.scalar.dma_start(out=ident, in_=ident_d.bitcast(F32R))
            rot_sb = consts.tile([P, P], F32R)
            nc.scalar.dma_start(out=rot_sb, in_=rot_t.bitcast(F32R))
            maska = consts.tile([P, 256], F32)
            nc.scalar.dma_start(out=maska, in_=ma_add)
            maskb = consts.tile([P, 256], F32)
            nc.scalar.dma_start(out=maskb, in_=mb_add)
            mula = consts.tile([P, 256], F32)
            nc.scalar.dma_start(out=mula, in_=ma_mul)
            mulb = consts.tile([P, 256], F32)
            nc.scalar.dma_start(out=mulb, in_=mb_mul)
            ones = consts.tile([P, 1], F32R)
            nc.scalar.dma_start(out=ones, in_=ones_d.bitcast(F32R))

            # persistent activations
            persist = ctx.enter_context(tc.tile_pool(name="persist", bufs=1))
            qT0 = persist.tile([P, T], F32R)     # head 0, [d, t], rope'd, pre-scaled
            qT1 = persist.tile([P, T], F32R)
            kT = persist.tile([P, T], F32R)
            v_sb = persist.tile([P, TB, D], F32R)  # v blocks [t_loc, tb, d]
            wo_sb = persist.tile([P, H_LOC, C], F32R)
            nc.scalar.dma_start(
                out=wo_sb, in_=wo.rearrange("(h p) n -> p h n", p=P).bitcast(F32R))

            # ---------------- phase P: transpose x, projections, rope -------
            with ExitStack() as pctx:
                wpool = pctx.enter_context(tc.tile_pool(name="wpool", bufs=1))
                wq_sb = wpool.tile([P, KC, H_LOC * D], F32R)
                nc.scalar.dma_start(
                    out=wq_sb,
                    in_=wq.rearrange("(kc p) n -> p kc n", p=P).bitcast(F32R))
                wk_sb = wpool.tile([P, KC, D], F32R)
                nc.scalar.dma_start(
                    out=wk_sb,
                    in_=wk.rearrange("(kc p) n -> p kc n", p=P).bitcast(F32R))
                wv_sb = wpool.tile([P, KC, D], F32R)
                nc.scalar.dma_start(
                    out=wv_sb,
                    in_=wv.rearrange("(kc p) n -> p kc n", p=P).bitcast(F32R))
                cos_sb = wpool.tile([P, T], F32)
                nc.scalar.dma_start(out=cos_sb, in_=cos_t)
                sin_sb = wpool.tile([P, T], F32)
                nc.scalar.dma_start(out=sin_sb, in_=sin_t)

                xstage = pctx.enter_context(tc.tile_pool(name="xstage", bufs=1))
                xtp = pctx.enter_context(tc.tile_pool(name="xtp", bufs=1))
                pst = pctx.enter_context(
                    tc.tile_pool(name="pst", bufs=2, space="PSUM"))
                ps_proj = pctx.enter_context(
                    tc.tile_pool(name="ps_proj", bufs=1, space="PSUM"))
                ps_rot = pctx.enter_context(
                    tc.tile_pool(name="ps_rot", bufs=2, space="PSUM"))
                raws = pctx.enter_context(tc.tile_pool(name="raws", bufs=2))

                for qq in range(QT):
                    tsl = slice(qq * 512, (qq + 1) * 512)
                    # load 4 x row-tiles for this quarter
                    x_ts = []
                    for j in range(4):
                        tb = qq * 4 + j
                        x_t = xstage.tile([P, C], F32R, tag=f"x{j}", bufs=1)
                        nc.sync.dma_start(
                            out=x_t,
                            in_=x[tb * P:(tb + 1) * P, :].bitcast(F32R))
                        x_ts.append(x_t)
                    # transpose into xT quarter [c-part, (kc, 512t)]
                    xT = xtp.tile([P, KC, 512], F32R, tag="xT")
                    for cb in range(KC):
                        pt = pst.tile([P, 512], F32R, tag="pt")
                        for j in range(4):
                            nc.tensor.transpose(
                                pt[:, j * P:(j + 1) * P],
                                x_ts[j][:, cb * P:(cb + 1) * P], ident)
                        if cb % 2 == 0:
                            nc.scalar.copy(xT[:, cb, :], pt)
                        else:
                            nc.vector.tensor_copy(xT[:, cb, :], pt)

                    # projections for this quarter
                    pq0 = ps_proj.tile([P, 512], F32, tag="pq0")
                    pq1 = ps_proj.tile([P, 512], F32, tag="pq1")
                    pk = ps_proj.tile([P, 512], F32, tag="pk")
                    pv = ps_proj.tile([P, 512], F32, tag="pv")
                    for kc in range(KC):
                        st = kc == 0
                        sp = kc == KC - 1
                        xr = xT[:, kc, :]
                        nc.tensor.matmul(
                            pq0, wq_sb[:, kc, 0:D], xr, start=st, stop=sp)
                        nc.tensor.matmul(
                            pq1, wq_sb[:, kc, D:2 * D], xr, start=st, stop=sp)
                        nc.tensor.matmul(
                            pk, wk_sb[:, kc, :], xr, start=st, stop=sp)
                        nc.tensor.matmul(
                            pv, wv_sb[:, kc, :], xr, start=st, stop=sp)

                    # v: copy to sbuf, then transpose to [t, d] blocks
                    vT_raw = raws.tile([P, 512], F32R, tag="vraw")
                    nc.scalar.copy(vT_raw, pv)
                    for j in range(4):
                        tb = qq * 4 + j
                        pt2 = pst.tile([P, 512], F32R, tag="pt")
                        nc.tensor.transpose(
                            pt2[:, 0:P], vT_raw[:, j * P:(j + 1) * P], ident)
                        nc.scalar.copy(v_sb[:, tb, :], pt2[:, 0:P])

                    # rope for q0, q1, k
                    for ps_raw, dst, tag in (
                            (pq0, qT0, "q0"), (pq1, qT1, "q1"), (pk, kT, "k")):
                        raw = raws.tile([P, 512], F32R, tag="raw", bufs=3)
                        nc.scalar.copy(raw, ps_raw)
                        prot = ps_rot.tile([P, 512], F32, tag="prot")
                        nc.tensor.matmul(
                            prot, rot_sb, raw, start=True, stop=True)
                        t1 = raws.tile([P, 512], F32, tag="t1")
                        nc.vector.tensor_mul(t1, prot, sin_sb[:, tsl])
                        t2 = raws.tile([P, 512], F32, tag="t2")
                        nc.vector.tensor_mul(t2, raw, cos_sb[:, tsl])
                        nc.vector.tensor_add(dst[:, tsl], t1, t2)

            # ---------------- phase A: attention + chunked o-proj ---------
            outT_pool = ctx.enter_context(tc.tile_pool(name="outT_pool", bufs=1))
            outT = [outT_pool.tile([P, T], F32R, name=f"outT{h}")
                    for h in range(H_LOC)]
            with ExitStack() as actx:
                epool = actx.enter_context(tc.tile_pool(name="epool", bufs=18))
                lpool = actx.enter_context(tc.tile_pool(name="lpool", bufs=3))
                spool = actx.enter_context(tc.tile_pool(name="spool", bufs=3))
                ostage = actx.enter_context(tc.tile_pool(name="ostage", bufs=4))
                ps_s = actx.enter_context(
                    tc.tile_pool(name="ps_s", bufs=2, space="PSUM"))
                ps_pv = actx.enter_context(
                    tc.tile_pool(name="ps_pv", bufs=1, space="PSUM"))
                ps_sum = actx.enter_context(
                    tc.tile_pool(name="ps_sum", bufs=1, space="PSUM"))
                ps_big = actx.enter_context(
                    tc.tile_pool(name="ps_big", bufs=2, space="PSUM"))

                for h in range(H_LOC):
                    qT = (qT0, qT1)[h]
                    for g in range(G):
                        qsl = slice(g * 256, (g + 1) * 256)
                        nkb = 2 * g + 2
                        kba = max(2 * g - 1, 0)
                        kbb = kba + 1
                        pog = ps_pv.tile([P, 256], F32, tag="pog")
                        pol = ps_pv.tile([P, 256], F32, tag="pol")
                        psg = ps_sum.tile([1, 256], F32, tag="psg")
                        psl = ps_sum.tile([1, 256], F32, tag="psl")
                        for kb in range(nkb):
                            ps = ps_s.tile([P, 256], F32, tag="ps")
                            nc.tensor.matmul(
                                ps, kT[:, kb * P:(kb + 1) * P],
                                qT[:, qsl], start=True, stop=True)
                            if kb == 2 * g:
                                nc.vector.tensor_add(ps, ps, maska)
                            elif kb == 2 * g + 1:
                                nc.vector.tensor_add(ps, ps, maskb)
                            e = epool.tile([P, 256], F32R, tag="e")
                            nc.scalar.activation(
                                e, ps, mybir.ActivationFunctionType.Exp)
                            st = kb == 0
                            sp = kb == nkb - 1
                            vr = v_sb[:, kb, :]
                            nc.tensor.matmul(pog, vr, e, start=st, stop=sp)
                            nc.tensor.matmul(psg, ones, e, start=st, stop=sp)
                            if kb in (kba, kbb):
                                first = kb == kba
                                msk = mula if first else mulb
                                el = lpool.tile([P, 256], F32R, tag="el")
                                nc.vector.tensor_mul(el, e, msk)
                                nc.tensor.matmul(
                                    pol, vr, el, start=first, stop=not first)
                                nc.tensor.matmul(
                                    psl, ones, el, start=first,
                                    stop=not first)
                        # normalize + combine for this group (0.5 folded
                        # into wo on host; broadcast 1/sum via 0-stride DMA)
                        rg = spool.tile([1, 256], F32, tag="rg")
                        rl = spool.tile([1, 256], F32, tag="rl")
                        nc.vector.reciprocal(rg, psg)
                        nc.vector.reciprocal(rl, psl)
                        bgs = lpool.tile([P, 256], F32, tag="bgs")
                        nc.gpsimd.partition_broadcast(bgs, rg)
                        bls = lpool.tile([P, 256], F32, tag="bls")
                        nc.gpsimd.partition_broadcast(bls, rl)
                        c1 = lpool.tile([P, 256], F32, tag="c1")
                        nc.vector.tensor_mul(c1, pog, bgs)
                        c2 = lpool.tile([P, 256], F32, tag="c2")
                        nc.vector.tensor_mul(c2, pol, bls)
                        nc.vector.tensor_add(outT[h][:, qsl], c1, c2)

                        if h == H_LOC - 1:
                            # o-proj for the two t-blocks this group covers
                            for tb in (2 * g, 2 * g + 1):
                                for cgi in range(4):
                                    csl = slice(cgi * 512, (cgi + 1) * 512)
                                    po = ps_big.tile([P, 512], F32, tag="big")
                                    for hh in range(H_LOC):
                                        nc.tensor.matmul(
                                            po,
                                            outT[hh][:, tb * P:(tb + 1) * P],
                                            wo_sb[:, hh, csl],
                                            start=(hh == 0),
                                            stop=(hh == H_LOC - 1))
                                    o_t = ostage.tile([P, 512], F32, tag="o_t")
                                    if cgi % 2 == 0:
                                        nc.scalar.copy(o_t, po)
                                    else:
                                        nc.vector.tensor_copy(o_t, po)
                                    nc.sync.dma_start(
                                        out=out[tb * P:(tb + 1) * P, csl],
                                        in_=o_t)
    return nc


def _build_program():
    if "nc" not in _PROGRAM_CACHE:
        nc = bacc.Bacc("TRN2", target_bir_lowering=False, debug=False,
                       num_devices=N_CORES)
        _emit(nc)
        nc.compile()
        _PROGRAM_CACHE["nc"] = nc
    return _PROGRAM_CACHE["nc"]


def _in_maps(x, wq, wk, wv, wo):
    x = np.ascontiguousarray(np.asarray(x, np.float32).reshape(T, C))
    wq = np.asarray(wq, np.float32)
    wk = np.asarray(wk, np.float32)
    wv = np.asarray(wv, np.float32)
    wo = np.asarray(wo, np.float32)
    consts = _host_constants()
    scale = 1.0 / math.sqrt(D)
    wq_s = wq * scale
    maps = []
    for c in range(N_CORES):
        h0 = H_LOC * c
        kv = h0 // (N_HEAD // N_KV_HEAD)
        m = {
            "x": x,
            "wq": np.ascontiguousarray(wq_s[:, h0 * D:(h0 + H_LOC) * D]),
            "wk": np.ascontiguousarray(wk[:, kv * D:(kv + 1) * D]),
            "wv": np.ascontiguousarray(wv[:, kv * D:(kv + 1) * D]),
            "wo": np.ascontiguousarray(wo[h0 * D:(h0 + H_LOC) * D, :] * 0.5),
        }
        m.update(consts)
        maps.append(m)
    return maps


def _run(inputs, trace=False):
    nc = _build_program()
    maps = _in_maps(inputs["x"], inputs["wq"], inputs["wk"],
                    inputs["wv"], inputs["wo"])
    res = run_bass_kernel_spmd(nc, maps, list(range(N_CORES)), trace=trace)
    total = np.zeros((T, C), np.float64)
    for rm in res.results:
        total += rm["out"].astype(np.float64)
    out = total.astype(np.float32).reshape(1, T, C)
    return out, res


def kernel(x, wq, wk, wv, wo):
    out, _ = _run({"x": x, "wq": wq, "wk": wk, "wv": wv, "wo": wo})
    return out



# revision 2
# speedup vs baseline: 1.0306x; 1.0306x over previous
"""DeepSeek-style sparse attention (causal + local-window softmax mix) on 8 trn2 cores.

v2: bf16 datapath + host-side x transpose + software-pipelined emission.

Sharding: tensor-parallel over heads. 16 Q heads / 4 KV heads; core c gets
Q heads {2c, 2c+1} and their shared KV head c//2. Each core computes a
partial output projection (its 256 rows of wo); the host sums the 8 partials.

Differences from v1:
- x is transposed and cast to bf16 on the HOST: no PE transposes, no
  PSUM->SBUF staging copies, half the x DMA traffic.
- All matmuls run in bf16 (same PE rate as fp32r at >=256-wide rhs, but
  1.0 cycles/row at any width, half the SBUF/DMA footprint).
- v is produced directly in [t, d] layout by using x^T chunks as the
  stationary operand (no v transposes).
- exp() runs on [128, 512/1024] PSUM spans (kb pairs/quads) to amortize
  the ~400-cycle Activation-engine access bubble.
- Attention emission is software-pipelined: score matmuls for chunk j+1
  are emitted before the PV/sum matmuls of chunk j, so the in-order PE
  never stalls on the Act-engine exp.
- Causal and local softmax share exp(S) tiles; sums via ones-matmuls into
  one [1,512] PSUM tile (causal|local), one reciprocal + one partition
  broadcast + a 512-wide multiply per group.
- Output partials are written bf16 (host sums in float64).
"""

import math

import numpy as np
import ml_dtypes

import concourse.bass as bass
import concourse.mybir as mybir
import concourse.tile as tile
from concourse import bacc
from concourse.bass_utils import run_bass_kernel_spmd

P = 128
T = 2048
C = 2048
D = 128                 # head dim
N_HEAD = 16
N_KV_HEAD = 4
H_LOC = 2               # q heads per core
N_CORES = 8
TB = T // P             # 16 t blocks of 128
KC = C // P             # 16 contraction chunks of 128
QT = T // 512           # 4 t quarters (projection phase)
G = T // 256            # 8 query groups of 256 (attention phase)
NEG = -1.0e30
F32 = mybir.dt.float32
BF16 = mybir.dt.bfloat16
BF = ml_dtypes.bfloat16

_PROGRAM_CACHE = {}


def _host_constants():
    """Host-precomputed constant tensors shipped as kernel inputs."""
    i = np.arange(P)
    tril = (i[:, None] <= i[None, :])          # [k_loc, q_loc]: valid iff k <= q
    mc_add = np.where(tril, 0.0, NEG).astype(np.float32)
    mc_mul = tril.astype(np.float32)
    ma_add = np.concatenate([mc_add, np.zeros((P, P), np.float32)], axis=1)
    mb_add = np.concatenate([np.full((P, P), NEG, np.float32), mc_add], axis=1)
    ma_mul = np.concatenate([mc_mul, np.ones((P, P), np.float32)], axis=1)
    mb_mul = np.concatenate([np.zeros((P, P), np.float32), mc_mul], axis=1)

    inv_freq = 1.0 / (10000.0 ** (np.arange(0, D, 2, dtype=np.float64) / D))
    t = np.arange(T, dtype=np.float64)
    freqs = t[:, None] * inv_freq[None, :]              # [T, D/2]
    emb = np.concatenate([freqs, freqs], axis=-1)       # [T, D]
    cos_t = np.cos(emb).T.astype(np.float32).copy()     # [D, T]
    sin_t = np.sin(emb).T.astype(np.float32).copy()

    # rot matmul weights: lhsT[d, d'] with out[d'] = -q[d'+64] (d'<64), q[d'-64] (d'>=64)
    rot_t = np.zeros((P, P), np.float32)
    rot_t[64 + np.arange(64), np.arange(64)] = -1.0
    rot_t[np.arange(64), 64 + np.arange(64)] = 1.0

    return {
        "mask_add": np.concatenate([ma_add, mb_add], axis=1),     # [P, 512] f32
        "mul_cat": np.concatenate([ma_mul, mb_mul], axis=1).astype(BF),
        "cos_b": cos_t.astype(BF), "sin_t": sin_t,
        "rot_t": rot_t.astype(BF),
        "ones_c": np.ones((P, 1), BF),
    }


def _emit(nc):
    # xf/wqf/wkvf are host-relaid-out so every DMA is long contiguous runs:
    # xf rows are SBUF partitions, cols are (kc, t) for one t-quarter
    xf = nc.dram_tensor("xf", [QT * P, KC * 512], BF16,
                        kind="ExternalInput").ap()
    wqf = nc.dram_tensor("wqf", [P, KC * H_LOC * D], BF16,
                         kind="ExternalInput").ap()
    wkvf = nc.dram_tensor("wkvf", [P, KC * 2 * D], BF16,
                          kind="ExternalInput").ap()
    wo = nc.dram_tensor("wo", [H_LOC * D, C], BF16, kind="ExternalInput").ap()
    cos_b_d = nc.dram_tensor("cos_b", [P, T], BF16, kind="ExternalInput").ap()
    sin_d = nc.dram_tensor("sin_t", [P, T], F32, kind="ExternalInput").ap()
    rot_d = nc.dram_tensor("rot_t", [P, P], BF16, kind="ExternalInput").ap()
    ones_d = nc.dram_tensor("ones_c", [P, 1], BF16, kind="ExternalInput").ap()
    mask_d = nc.dram_tensor("mask_add", [P, 512], F32, kind="ExternalInput").ap()
    mul_d = nc.dram_tensor("mul_cat", [P, 512], BF16, kind="ExternalInput").ap()
    out = nc.dram_tensor("out", [T, C], BF16, kind="ExternalOutput").ap()

    Exp = mybir.ActivationFunctionType.Exp

    with tile.TileContext(nc) as tc:
        from contextlib import ExitStack
        with ExitStack() as ctx:
            # DMA queue plan: projection weights FIRST on the scalar queue
            # (they gate the first PE matmul), small consts after; cos/sin
            # (needed ~10us in) then wo (needed ~halfway) on the vector
            # queue; x chunks + out stores on the sync queue.
            consts = ctx.enter_context(tc.tile_pool(name="consts", bufs=1))
            # one TILE per 4-kc chunk: tile-granular dependency tracking
            # means a single big tile would gate the first matmul on the
            # last chunk's DMA
            wq_t = []
            wkv_t = []
            kc2t = {}
            bounds = [(0, 1), (1, 5), (5, 9), (9, 13), (13, 16)]
            for ti, (lo_kc, hi_kc) in enumerate(bounds):
                n = hi_kc - lo_kc
                wqc = consts.tile([P, n, H_LOC * D], BF16, name=f"wq{ti}")
                nc.scalar.dma_start(
                    out=wqc, in_=wqf[:, lo_kc * 256:hi_kc * 256])
                wq_t.append(wqc)
                wkvc = consts.tile([P, n, 2 * D], BF16, name=f"wkv{ti}")
                nc.scalar.dma_start(
                    out=wkvc, in_=wkvf[:, lo_kc * 256:hi_kc * 256])
                wkv_t.append(wkvc)
                for kc in range(lo_kc, hi_kc):
                    kc2t[kc] = (ti, kc - lo_kc)

            def wq_sl(kc, lo, hi):
                ti, off = kc2t[kc]
                return wq_t[ti][:, off, lo:hi]

            def wkv_sl(kc, lo, hi):
                ti, off = kc2t[kc]
                return wkv_t[ti][:, off, lo:hi]
            rot_sb = consts.tile([P, P], BF16)
            nc.scalar.dma_start(out=rot_sb, in_=rot_d)
            ones = consts.tile([P, 1], BF16)
            nc.scalar.dma_start(out=ones, in_=ones_d)
            maskc = consts.tile([P, 512], F32)
            nc.scalar.dma_start(out=maskc, in_=mask_d)
            mulc = consts.tile([P, 512], BF16)
            nc.scalar.dma_start(out=mulc, in_=mul_d)
            cos_sb = consts.tile([P, T], BF16)
            nc.scalar.dma_start(out=cos_sb, in_=cos_b_d)
            sin_sb = consts.tile([P, T], F32)
            nc.scalar.dma_start(out=sin_sb, in_=sin_d)

            # persistent activations, split per quarter/group so readers
            # depend only on the slice they touch (tile-granular deps)
            persist = ctx.enter_context(tc.tile_pool(name="persist", bufs=1))
            qT_t = [[persist.tile([P, 512], BF16, name=f"qT{h}_{q}")
                     for q in range(QT)] for h in range(H_LOC)]
            kT_t = [persist.tile([P, 512], BF16, name=f"kT_{q}")
                    for q in range(QT)]
            v_t = [persist.tile([P, 512], BF16, name=f"v_{q}")
                   for q in range(QT)]
            wo_sb = persist.tile([P, H_LOC, C], BF16)
            nc.scalar.dma_start(
                out=wo_sb, in_=wo.rearrange("(h p) n -> p h n", p=P))
            outT_t = [[persist.tile([P, 256], BF16, name=f"oT{h}_{g}")
                       for g in range(G)] for h in range(H_LOC)]

            def kT_sl(kb):
                return kT_t[kb // 4][:, (kb % 4) * P:(kb % 4 + 1) * P]

            def v_sl(kb):
                return v_t[kb // 4][:, (kb % 4) * D:(kb % 4 + 1) * D]

            def qT_sl(h, g):
                return qT_t[h][g // 2][:, (g % 2) * 256:(g % 2 + 1) * 256]

            def outT_sl(h, tb):
                return outT_t[h][tb // 2][:, (tb % 2) * P:(tb % 2 + 1) * P]

            # ---------------- phase P: projections + rope -------------------
            with ExitStack() as pctx:
                xpool = pctx.enter_context(tc.tile_pool(name="xpool", bufs=2))
                ps_proj = pctx.enter_context(
                    tc.tile_pool(name="ps_proj", bufs=1, space="PSUM"))
                ps_rot = pctx.enter_context(
                    tc.tile_pool(name="ps_rot", bufs=2, space="PSUM"))
                raws = pctx.enter_context(tc.tile_pool(name="raws", bufs=3))
                t12 = pctx.enter_context(tc.tile_pool(name="t12", bufs=2))

                def rope(ps_raw, dst, tsl):
                    raw = raws.tile([P, 512], BF16, tag="raw", name="raw")
                    nc.scalar.copy(raw, ps_raw)
                    prot = ps_rot.tile([P, 512], F32, tag="prot", name="prot")
                    nc.tensor.matmul(prot, rot_sb, raw, start=True, stop=True)
                    t1 = t12.tile([P, 512], F32, tag="t1", name="t1")
                    nc.vector.tensor_mul(t1, prot, sin_sb[:, tsl])
                    t2 = t12.tile([P, 512], F32, tag="t2", name="t2")
                    nc.vector.tensor_mul(t2, raw, cos_sb[:, tsl])
                    nc.vector.tensor_add(dst, t1, t2)

                def load_xq(qq):
                    xq = []
                    rows = slice(qq * P, (qq + 1) * P)
                    for j4 in range(4):
                        xc = xpool.tile([P, 4, 512], BF16, tag=f"xq{j4}",
                                        name=f"xq{j4}")
                        nc.sync.dma_start(
                            out=xc, in_=xf[rows, j4 * 2048:(j4 + 1) * 2048])
                        xq.append(xc)
                    return xq

                def xq_sl(xq, kc):
                    return xq[kc // 4][:, kc % 4, :]

                def xq_slj(xq, kc, jsl):
                    return xq[kc // 4][:, kc % 4, jsl]

                def proj_tiles():
                    pq0 = ps_proj.tile([P, 512], F32, tag="pq0", name="pq0")
                    pq1 = ps_proj.tile([P, 512], F32, tag="pq1", name="pq1")
                    pk = ps_proj.tile([P, 512], F32, tag="pk", name="pk")
                    pv = ps_proj.tile([P, 512], F32, tag="pv", name="pv")
                    return pq0, pq1, pk, pv

                def emit_prev_rope(prev, i):
                    # rope/v-copy of the previous quarter's tensor i, emitted
                    # just before this quarter's chain reuses its PSUM bank
                    if prev is None:
                        return
                    pq0, pq1, pk, pv, ptsl, pqq = prev
                    if i == 0:   # kT first: phase A's first scores need it
                        rope(pk, kT_t[pqq], ptsl)
                    elif i == 1:
                        rope(pq0, qT_t[0][pqq], ptsl)
                    elif i == 2:
                        nc.scalar.copy(v_t[pqq], pv)
                    else:
                        rope(pq1, qT_t[1][pqq], ptsl)

                # quarter 0: kc-interleaved so PE consumption matches x DMA
                # arrival order (no pstate-resetting stalls at warmup)
                xq = load_xq(0)
                pq0, pq1, pk, pv = proj_tiles()
                for kc in range(KC):
                    st, sp = kc == 0, kc == KC - 1
                    nc.tensor.matmul(pq0, wq_sl(kc, 0, D), xq_sl(xq, kc),
                                     start=st, stop=sp)
                    nc.tensor.matmul(pq1, wq_sl(kc, D, 2 * D),
                                     xq_sl(xq, kc), start=st, stop=sp)
                    nc.tensor.matmul(pk, wkv_sl(kc, 0, D), xq_sl(xq, kc),
                                     start=st, stop=sp)
                    # v directly in [t, d] layout: x^T chunk stationary.
                    # all 4 j-blocks share one PSUM bank: only (kc0, j0)
                    # starts the zero region, only (kc15, j3) stops it
                    for j in range(4):
                        jsl = slice(j * P, (j + 1) * P)
                        nc.tensor.matmul(pv[:, jsl], xq_slj(xq, kc, jsl),
                                         wkv_sl(kc, D, 2 * D),
                                         start=(st and j == 0),
                                         stop=(sp and j == 3))
                prev = (pq0, pq1, pk, pv, slice(0, 512), 0)

                # quarters 1-3: per-tensor chains, interleaved with the
                # previous quarter's rope/v-copy (whose PSUM banks they reuse)
                for qq in range(1, QT):
                    tsl = slice(qq * 512, (qq + 1) * 512)
                    xq = load_xq(qq)
                    emit_prev_rope(prev, 0)
                    pq0, pq1, pk, pv = proj_tiles()
                    for kc in range(KC):
                        nc.tensor.matmul(pk, wkv_sl(kc, 0, D), xq_sl(xq, kc),
                                         start=(kc == 0), stop=(kc == KC - 1))
                    emit_prev_rope(prev, 1)
                    for kc in range(KC):
                        nc.tensor.matmul(pq0, wq_sl(kc, 0, D), xq_sl(xq, kc),
                                         start=(kc == 0), stop=(kc == KC - 1))
                    if qq == QT - 1:
                        # last quarter: rope its own kT mid-quarter so phase
                        # A's first score matmuls aren't gated on the tail
                        rope(pk, kT_t[qq], tsl)
                    else:
                        emit_prev_rope(prev, 2)
                    for kc in range(KC):
                        for j in range(4):
                            jsl = slice(j * P, (j + 1) * P)
                            nc.tensor.matmul(pv[:, jsl], xq_slj(xq, kc, jsl),
                                             wkv_sl(kc, D, 2 * D),
                                             start=(kc == 0 and j == 0),
                                             stop=(kc == KC - 1 and j == 3))
                    if qq == QT - 1:
                        emit_prev_rope(prev, 2)
                        rope(pq0, qT_t[0][qq], tsl)
                    else:
                        emit_prev_rope(prev, 3)
                    for kc in range(KC):
                        nc.tensor.matmul(pq1, wq_sl(kc, D, 2 * D),
                                         xq_sl(xq, kc),
                                         start=(kc == 0), stop=(kc == KC - 1))
                    if qq == QT - 1:
                        emit_prev_rope(prev, 3)
                        nc.scalar.copy(v_t[qq], pv)
                        rope(pq1, qT_t[1][qq], tsl)
                    prev = (pq0, pq1, pk, pv, tsl, qq)

            # ---------------- phase A: attention + chunked o-proj -----------
            with ExitStack() as actx:
                ps_pool = actx.enter_context(
                    tc.tile_pool(name="ps_pool", bufs=2, space="PSUM"))
                pv_pool = actx.enter_context(
                    tc.tile_pool(name="pv_pool", bufs=2, space="PSUM"))
                sums_pool = actx.enter_context(
                    tc.tile_pool(name="sums_pool", bufs=1, space="PSUM"))
                po_pool = actx.enter_context(
                    tc.tile_pool(name="po_pool", bufs=1, space="PSUM"))
                epool = actx.enter_context(tc.tile_pool(name="epool", bufs=4))
                elpool = actx.enter_context(tc.tile_pool(name="elpool", bufs=3))
                spool = actx.enter_context(tc.tile_pool(name="spool", bufs=2))
                bpool = actx.enter_context(tc.tile_pool(name="bpool", bufs=2))
                mpool = actx.enter_context(tc.tile_pool(name="mpool", bufs=2))
                opool = actx.enter_context(tc.tile_pool(name="opool", bufs=4))

                # build the flat chunk-job list: g outer, h inner
                class Grp:
                    pass

                jobs = []
                for g in range(G):
                    for h in range(H_LOC):
                        nkb = 2 * g + 2
                        grp = Grp()
                        grp.g, grp.h, grp.nkb = g, h, nkb
                        grp.kba = max(2 * g - 1, 0)
                        grp.kbb = grp.kba + 1
                        grp.e_map = {}
                        grp.pvacc = None
                        grp.sums = None
                        chunks = []
                        i = 0
                        while nkb - i >= 4:
                            chunks.append(list(range(i, i + 4)))
                            i += 4
                        if i < nkb:
                            chunks.append(list(range(i, i + 2)))
                        for ci, chunk in enumerate(chunks):
                            jobs.append((grp, chunk, ci == 0,
                                         ci == len(chunks) - 1))

                oproj_q = []
                copy_engines = [nc.scalar.copy, nc.vector.tensor_copy]
                copy_i = [0]

                def emit_oproj(tb, cgi, from_ps=False):
                    csl = slice(cgi * 512, (cgi + 1) * 512)
                    if from_ps:
                        # epilogue: score-chunk PSUM banks are dead, rotate po
                        # through them so chains overlap their free-up copies
                        po = ps_pool.tile([P, 1024], F32, tag="ps",
                                          name="ps")[:, 0:512]
                    else:
                        po = po_pool.tile([P, 512], F32, tag="po", name="po")
                    for hh in range(H_LOC):
                        nc.tensor.matmul(po, outT_sl(hh, tb),
                                         wo_sb[:, hh, csl],
                                         start=(hh == 0),
                                         stop=(hh == H_LOC - 1))
                    o_t = opool.tile([P, 512], BF16, tag="o_t", name="o_t")
                    copy_engines[copy_i[0] % 2](o_t, po)
                    copy_i[0] += 1
                    nc.sync.dma_start(
                        out=out[tb * P:(tb + 1) * P, csl], in_=o_t)

                def drain_oproj(n, from_ps=False):
                    for i in range(min(n, len(oproj_q))):
                        emit_oproj(*oproj_q.pop(0), from_ps=from_ps)

                def emit_scores(job):
                    grp, chunk, first, last = job
                    g, h = grp.g, grp.h
                    qsl = slice(g * 256, (g + 1) * 256)
                    if first:
                        grp.pvacc = pv_pool.tile([P, 512], F32, tag="pvacc",
                                                 name="pvacc")
                        grp.sums = sums_pool.tile([1, 512], F32, tag="sums",
                                                  name="sums")
                    ps = ps_pool.tile([P, 1024], F32, tag="ps", name="ps")
                    w = len(chunk) * 256
                    # two 256-col regions share a 2KB PSUM bank: the first
                    # matmul's start=True lazy-zeroes the bank, the second
                    # writes its half via the pending-zero path
                    for i, kb in enumerate(chunk):
                        nc.tensor.matmul(ps[:, i * 256:(i + 1) * 256],
                                         kT_sl(kb), qT_sl(h, g),
                                         start=(i % 2 == 0), stop=(i % 2 == 1))
                    if last:
                        off = w - 512
                        nc.vector.tensor_add(ps[:, off:off + 512],
                                             ps[:, off:off + 512], maskc)
                    e = epool.tile([P, 1024], BF16, tag="e", name="e")
                    nc.scalar.activation(e[:, 0:w], ps[:, 0:w], Exp)
                    for i, kb in enumerate(chunk):
                        grp.e_map[kb] = (e, i * 256)

                def emit_consume(job):
                    grp, chunk, first, last = job
                    g, h, nkb = grp.g, grp.h, grp.nkb
                    pvacc, sums = grp.pvacc, grp.sums
                    for i, kb in enumerate(chunk):
                        e, off = grp.e_map[kb]
                        esl = e[:, off:off + 256]
                        vr = v_sl(kb)
                        nc.tensor.matmul(pvacc[:, 0:256], vr, esl,
                                         start=(kb == 0),
                                         stop=(kb == nkb - 1))
                        nc.tensor.matmul(sums[0:1, 0:256], ones, esl,
                                         start=(kb == 0),
                                         stop=(kb == nkb - 1))
                        if kb in (grp.kba, grp.kbb):
                            # pol/psl share the pog/psg PSUM banks: they ride
                            # on the pending-zero set by pog/psg's start=True
                            # (start=False write-then-accumulate semantics)
                            wi = 0 if kb == grp.kba else 1
                            el = elpool.tile([P, 256], BF16, tag="el",
                                             name="el")
                            nc.vector.tensor_mul(
                                el, esl, mulc[:, wi * 256:(wi + 1) * 256])
                            nc.tensor.matmul(pvacc[:, 256:512], vr, el,
                                             start=False, stop=False,
                                             skip_group_check=True)
                            nc.tensor.matmul(sums[0:1, 256:512], ones, el,
                                             start=False, stop=False,
                                             skip_group_check=True)
                    if last:
                        qsl = slice(g * 256, (g + 1) * 256)
                        rec = spool.tile([1, 512], F32, tag="rec", name="rec")
                        nc.vector.reciprocal(rec, sums)
                        bc = bpool.tile([P, 512], F32, tag="bc", name="bc")
                        nc.gpsimd.partition_broadcast(bc, rec)
                        m1 = mpool.tile([P, 512], F32, tag="m1", name="m1")
                        nc.vector.tensor_mul(m1, pvacc, bc)
                        nc.vector.tensor_add(outT_t[h][g],
                                             m1[:, 0:256], m1[:, 256:512])
                        if h == H_LOC - 1:
                            for tb in (2 * g, 2 * g + 1):
                                for cgi in range(4):
                                    oproj_q.append((tb, cgi))

                emit_scores(jobs[0])
                for j in range(1, len(jobs)):
                    emit_scores(jobs[j])
                    # split the o-proj drain around the consume stage so two
                    # po chains never sit back-to-back on the in-order PE
                    # (the second would stall on the first's PSUM-free copy)
                    drain_oproj(1)
                    emit_consume(jobs[j - 1])
                    drain_oproj(1, from_ps=True)
                emit_consume(jobs[-1])
                while oproj_q:
                    drain_oproj(1)
                    drain_oproj(1, from_ps=True)
    return nc


def _build_program():
    if "nc" not in _PROGRAM_CACHE:
        nc = bacc.Bacc("TRN2", target_bir_lowering=False, debug=False,
                       num_devices=N_CORES)
        _emit(nc)
        nc.compile()
        _PROGRAM_CACHE["nc"] = nc
    return _PROGRAM_CACHE["nc"]


def _feed_layout(w):
    """[C, n] weight -> [P, KC * n] with rows = SBUF partitions."""
    n = w.shape[1]
    return np.ascontiguousarray(
        w.reshape(KC, P, n).transpose(1, 0, 2).reshape(P, KC * n))


def _in_maps(x, wq, wk, wv, wo):
    x = np.asarray(x, np.float32).reshape(T, C)
    xT = x.T.astype(BF)
    # xf[q*P + p, kc*512 + t] = xT[kc*128 + p, q*512 + t]
    xf = np.ascontiguousarray(
        xT.reshape(KC, P, QT, 512).transpose(2, 1, 0, 3).reshape(
            QT * P, KC * 512))
    wq = np.asarray(wq, np.float32)
    wk = np.asarray(wk, np.float32)
    wv = np.asarray(wv, np.float32)
    wo = np.asarray(wo, np.float32)
    consts = _host_constants()
    scale = 1.0 / math.sqrt(D)
    wq_s = wq * scale
    maps = []
    for c in range(N_CORES):
        h0 = H_LOC * c
        kv = h0 // (N_HEAD // N_KV_HEAD)
        m = {
            "xf": xf,
            "wqf": _feed_layout(
                wq_s[:, h0 * D:(h0 + H_LOC) * D].astype(BF)),
            "wkvf": _feed_layout(np.concatenate(
                [wk[:, kv * D:(kv + 1) * D], wv[:, kv * D:(kv + 1) * D]],
                axis=1).astype(BF)),
            "wo": np.ascontiguousarray(
                wo[h0 * D:(h0 + H_LOC) * D, :] * 0.5).astype(BF),
        }
        m.update(consts)
        maps.append(m)
    return maps


def _run(inputs, trace=False):
    nc = _build_program()
    maps = _in_maps(inputs["x"], inputs["wq"], inputs["wk"],
                    inputs["wv"], inputs["wo"])
    res = run_bass_kernel_spmd(nc, maps, list(range(N_CORES)), trace=trace)
    total = np.zeros((T, C), np.float64)
    for rm in res.results:
        total += rm["out"].astype(np.float64)
    out = total.astype(np.float32).reshape(1, T, C)
    return out, res


def kernel(x, wq, wk, wv, wo):
    out, _ = _run({"x": x, "wq": wq, "wk": wk, "wv": wv, "wo": wo})
    return out


# revision 3
# speedup vs baseline: 1.0731x; 1.0412x over previous
"""DeepSeek-style sparse attention (causal + local-window softmax mix) on 8 trn2 cores.

v2: bf16 datapath + host-side x transpose + software-pipelined emission.

Sharding: tensor-parallel over heads. 16 Q heads / 4 KV heads; core c gets
Q heads {2c, 2c+1} and their shared KV head c//2. Each core computes a
partial output projection (its 256 rows of wo); the host sums the 8 partials.

Differences from v1:
- x is transposed and cast to bf16 on the HOST: no PE transposes, no
  PSUM->SBUF staging copies, half the x DMA traffic.
- All matmuls run in bf16 (same PE rate as fp32r at >=256-wide rhs, but
  1.0 cycles/row at any width, half the SBUF/DMA footprint).
- v is produced directly in [t, d] layout by using x^T chunks as the
  stationary operand (no v transposes).
- exp() runs on [128, 512/1024] PSUM spans (kb pairs/quads) to amortize
  the ~400-cycle Activation-engine access bubble.
- Attention emission is software-pipelined: score matmuls for chunk j+1
  are emitted before the PV/sum matmuls of chunk j, so the in-order PE
  never stalls on the Act-engine exp.
- Causal and local softmax share exp(S) tiles; sums via ones-matmuls into
  one [1,512] PSUM tile (causal|local), one reciprocal + one partition
  broadcast + a 512-wide multiply per group.
- Output partials are written bf16 (host sums in float64).
"""

import math

import numpy as np
import ml_dtypes

import concourse.bass as bass
import concourse.mybir as mybir
import concourse.tile as tile
from concourse import bacc
from concourse.bass_utils import run_bass_kernel_spmd

P = 128
T = 2048
C = 2048
D = 128                 # head dim
N_HEAD = 16
N_KV_HEAD = 4
H_LOC = 2               # q heads per core
N_CORES = 8
TB = T // P             # 16 t blocks of 128
KC = C // P             # 16 contraction chunks of 128
QT = T // 512           # 4 t quarters (projection phase)
G = T // 256            # 8 query groups of 256 (attention phase)
NEG = -1.0e30
F32 = mybir.dt.float32
BF16 = mybir.dt.bfloat16
BF = ml_dtypes.bfloat16

_PROGRAM_CACHE = {}


def _host_constants():
    """Host-precomputed constant tensors shipped as kernel inputs."""
    i = np.arange(P)
    tril = (i[:, None] <= i[None, :])          # [k_loc, q_loc]: valid iff k <= q
    mc_add = np.where(tril, 0.0, NEG).astype(np.float32)
    mc_mul = tril.astype(np.float32)
    ma_add = np.concatenate([mc_add, np.zeros((P, P), np.float32)], axis=1)
    mb_add = np.concatenate([np.full((P, P), NEG, np.float32), mc_add], axis=1)
    ma_mul = np.concatenate([mc_mul, np.ones((P, P), np.float32)], axis=1)
    mb_mul = np.concatenate([np.zeros((P, P), np.float32), mc_mul], axis=1)

    inv_freq = 1.0 / (10000.0 ** (np.arange(0, D, 2, dtype=np.float64) / D))
    t = np.arange(T, dtype=np.float64)
    freqs = t[:, None] * inv_freq[None, :]              # [T, D/2]
    emb = np.concatenate([freqs, freqs], axis=-1)       # [T, D]
    cos_t = np.cos(emb).T.astype(np.float32).copy()     # [D, T]
    sin_t = np.sin(emb).T.astype(np.float32).copy()

    # rot matmul weights: lhsT[d, d'] with out[d'] = -q[d'+64] (d'<64), q[d'-64] (d'>=64)
    rot_t = np.zeros((P, P), np.float32)
    rot_t[64 + np.arange(64), np.arange(64)] = -1.0
    rot_t[np.arange(64), 64 + np.arange(64)] = 1.0

    return {
        "mask_add": np.concatenate([ma_add, mc_add], axis=1),     # [P, 384] f32
        "mul_cat": np.concatenate([ma_mul, mb_mul], axis=1).astype(BF),
        "cos_b": cos_t.astype(BF), "sin_t": sin_t,
        "rot_t": rot_t.astype(BF),
        "ones_c": np.ones((P, 1), BF),
    }


def _emit(nc):
    # xf/wqf/wkvf are host-relaid-out so every DMA is long contiguous runs:
    # xf rows are SBUF partitions, cols are (kc, t) for one t-quarter
    xf = nc.dram_tensor("xf", [QT * P, KC * 512], BF16,
                        kind="ExternalInput").ap()
    wqf = nc.dram_tensor("wqf", [P, KC * H_LOC * D], BF16,
                         kind="ExternalInput").ap()
    wkvf = nc.dram_tensor("wkvf", [P, KC * 2 * D], BF16,
                          kind="ExternalInput").ap()
    wo = nc.dram_tensor("wo", [H_LOC * D, C], BF16, kind="ExternalInput").ap()
    cos_b_d = nc.dram_tensor("cos_b", [P, T], BF16, kind="ExternalInput").ap()
    sin_d = nc.dram_tensor("sin_t", [P, T], F32, kind="ExternalInput").ap()
    rot_d = nc.dram_tensor("rot_t", [P, P], BF16, kind="ExternalInput").ap()
    ones_d = nc.dram_tensor("ones_c", [P, 1], BF16, kind="ExternalInput").ap()
    mask_d = nc.dram_tensor("mask_add", [P, 384], F32, kind="ExternalInput").ap()
    mul_d = nc.dram_tensor("mul_cat", [P, 512], BF16, kind="ExternalInput").ap()
    out = nc.dram_tensor("out", [T, C], BF16, kind="ExternalOutput").ap()

    Exp = mybir.ActivationFunctionType.Exp

    with tile.TileContext(nc) as tc:
        from contextlib import ExitStack
        with ExitStack() as ctx:
            # DMA queue plan: projection weights FIRST on the scalar queue
            # (they gate the first PE matmul), small consts after; cos/sin
            # (needed ~10us in) then wo (needed ~halfway) on the vector
            # queue; x chunks + out stores on the sync queue.
            consts = ctx.enter_context(tc.tile_pool(name="consts", bufs=1))
            # one TILE per 4-kc chunk: tile-granular dependency tracking
            # means a single big tile would gate the first matmul on the
            # last chunk's DMA
            wq_t = []
            wkv_t = []
            kc2t = {}
            bounds = [(0, 1), (1, 5), (5, 9), (9, 13), (13, 16)]
            for ti, (lo_kc, hi_kc) in enumerate(bounds):
                n = hi_kc - lo_kc
                wqc = consts.tile([P, n, H_LOC * D], BF16, name=f"wq{ti}")
                nc.scalar.dma_start(
                    out=wqc, in_=wqf[:, lo_kc * 256:hi_kc * 256])
                wq_t.append(wqc)
                wkvc = consts.tile([P, n, 2 * D], BF16, name=f"wkv{ti}")
                nc.scalar.dma_start(
                    out=wkvc, in_=wkvf[:, lo_kc * 256:hi_kc * 256])
                wkv_t.append(wkvc)
                for kc in range(lo_kc, hi_kc):
                    kc2t[kc] = (ti, kc - lo_kc)

            def wq_sl(kc, lo, hi):
                ti, off = kc2t[kc]
                return wq_t[ti][:, off, lo:hi]

            def wkv_sl(kc, lo, hi):
                ti, off = kc2t[kc]
                return wkv_t[ti][:, off, lo:hi]
            rot_sb = consts.tile([P, P], BF16)
            nc.scalar.dma_start(out=rot_sb, in_=rot_d)
            ones = consts.tile([P, 1], BF16)
            nc.scalar.dma_start(out=ones, in_=ones_d)
            maskc = consts.tile([P, 384], F32)
            nc.scalar.dma_start(out=maskc, in_=mask_d)
            mulc = consts.tile([P, 512], BF16)
            nc.scalar.dma_start(out=mulc, in_=mul_d)
            cos_sb = consts.tile([P, T], BF16)
            nc.scalar.dma_start(out=cos_sb, in_=cos_b_d)
            sin_sb = consts.tile([P, T], F32)
            nc.scalar.dma_start(out=sin_sb, in_=sin_d)

            # persistent activations, split per quarter/group so readers
            # depend only on the slice they touch (tile-granular deps)
            persist = ctx.enter_context(tc.tile_pool(name="persist", bufs=1))
            qT_t = [[persist.tile([P, 512], BF16, name=f"qT{h}_{q}")
                     for q in range(QT)] for h in range(H_LOC)]
            kT_t = [persist.tile([P, 512], BF16, name=f"kT_{q}")
                    for q in range(QT)]
            v_t = [persist.tile([P, 512], BF16, name=f"v_{q}")
                   for q in range(QT)]
            wo_sb = persist.tile([P, H_LOC, C], BF16)
            nc.scalar.dma_start(
                out=wo_sb, in_=wo.rearrange("(h p) n -> p h n", p=P))
            outT_t = [[persist.tile([P, 256], BF16, name=f"oT{h}_{g}")
                       for g in range(G)] for h in range(H_LOC)]

            def kT_sl(kb):
                return kT_t[kb // 4][:, (kb % 4) * P:(kb % 4 + 1) * P]

            def v_sl(kb):
                return v_t[kb // 4][:, (kb % 4) * D:(kb % 4 + 1) * D]

            def qT_sl(h, g):
                return qT_t[h][g // 2][:, (g % 2) * 256:(g % 2 + 1) * 256]

            def outT_sl(h, tb):
                return outT_t[h][tb // 2][:, (tb % 2) * P:(tb % 2 + 1) * P]

            # ---------------- phase P: projections + rope -------------------
            with ExitStack() as pctx:
                xpool = pctx.enter_context(tc.tile_pool(name="xpool", bufs=2))
                ps_proj = pctx.enter_context(
                    tc.tile_pool(name="ps_proj", bufs=1, space="PSUM"))
                ps_rot = pctx.enter_context(
                    tc.tile_pool(name="ps_rot", bufs=2, space="PSUM"))
                raws = pctx.enter_context(tc.tile_pool(name="raws", bufs=3))
                t12 = pctx.enter_context(tc.tile_pool(name="t12", bufs=2))

                def rope(ps_raw, dst, tsl):
                    raw = raws.tile([P, 512], BF16, tag="raw", name="raw")
                    nc.scalar.copy(raw, ps_raw)
                    prot = ps_rot.tile([P, 512], F32, tag="prot", name="prot")
                    nc.tensor.matmul(prot, rot_sb, raw, start=True, stop=True)
                    t1 = t12.tile([P, 512], F32, tag="t1", name="t1")
                    nc.vector.tensor_mul(t1, prot, sin_sb[:, tsl])
                    t2 = t12.tile([P, 512], F32, tag="t2", name="t2")
                    nc.vector.tensor_mul(t2, raw, cos_sb[:, tsl])
                    nc.vector.tensor_add(dst, t1, t2)

                def load_xq(qq):
                    xq = []
                    rows = slice(qq * P, (qq + 1) * P)
                    for j4 in range(4):
                        xc = xpool.tile([P, 4, 512], BF16, tag=f"xq{j4}",
                                        name=f"xq{j4}")
                        nc.sync.dma_start(
                            out=xc, in_=xf[rows, j4 * 2048:(j4 + 1) * 2048])
                        xq.append(xc)
                    return xq

                def xq_sl(xq, kc):
                    return xq[kc // 4][:, kc % 4, :]

                def xq_slj(xq, kc, jsl):
                    return xq[kc // 4][:, kc % 4, jsl]

                def proj_tiles():
                    pq0 = ps_proj.tile([P, 512], F32, tag="pq0", name="pq0")
                    pq1 = ps_proj.tile([P, 512], F32, tag="pq1", name="pq1")
                    pk = ps_proj.tile([P, 512], F32, tag="pk", name="pk")
                    pv = ps_proj.tile([P, 512], F32, tag="pv", name="pv")
                    return pq0, pq1, pk, pv

                def emit_prev_rope(prev, i):
                    # rope/v-copy of the previous quarter's tensor i, emitted
                    # just before this quarter's chain reuses its PSUM bank
                    if prev is None:
                        return
                    pq0, pq1, pk, pv, ptsl, pqq = prev
                    if i == 0:   # kT first: phase A's first scores need it
                        rope(pk, kT_t[pqq], ptsl)
                    elif i == 1:
                        rope(pq0, qT_t[0][pqq], ptsl)
                    elif i == 2:
                        nc.scalar.copy(v_t[pqq], pv)
                    else:
                        rope(pq1, qT_t[1][pqq], ptsl)

                # quarter 0: kc-interleaved so PE consumption matches x DMA
                # arrival order (no pstate-resetting stalls at warmup)
                xq = load_xq(0)
                pq0, pq1, pk, pv = proj_tiles()
                for kc in range(KC):
                    st, sp = kc == 0, kc == KC - 1
                    nc.tensor.matmul(pq0, wq_sl(kc, 0, D), xq_sl(xq, kc),
                                     start=st, stop=sp)
                    nc.tensor.matmul(pq1, wq_sl(kc, D, 2 * D),
                                     xq_sl(xq, kc), start=st, stop=sp)
                    nc.tensor.matmul(pk, wkv_sl(kc, 0, D), xq_sl(xq, kc),
                                     start=st, stop=sp)
                    # v directly in [t, d] layout: x^T chunk stationary.
                    # all 4 j-blocks share one PSUM bank: only (kc0, j0)
                    # starts the zero region, only (kc15, j3) stops it
                    for j in range(4):
                        jsl = slice(j * P, (j + 1) * P)
                        nc.tensor.matmul(pv[:, jsl], xq_slj(xq, kc, jsl),
                                         wkv_sl(kc, D, 2 * D),
                                         start=(st and j == 0),
                                         stop=(sp and j == 3))
                prev = (pq0, pq1, pk, pv, slice(0, 512), 0)

                # quarters 1-3: per-tensor chains, interleaved with the
                # previous quarter's rope/v-copy (whose PSUM banks they reuse)
                for qq in range(1, QT):
                    tsl = slice(qq * 512, (qq + 1) * 512)
                    xq = load_xq(qq)
                    emit_prev_rope(prev, 0)
                    pq0, pq1, pk, pv = proj_tiles()
                    for kc in range(KC):
                        nc.tensor.matmul(pk, wkv_sl(kc, 0, D), xq_sl(xq, kc),
                                         start=(kc == 0), stop=(kc == KC - 1))
                    emit_prev_rope(prev, 1)
                    for kc in range(KC):
                        nc.tensor.matmul(pq0, wq_sl(kc, 0, D), xq_sl(xq, kc),
                                         start=(kc == 0), stop=(kc == KC - 1))
                    if qq == QT - 1:
                        # last quarter: rope its own kT mid-quarter so phase
                        # A's first score matmuls aren't gated on the tail
                        rope(pk, kT_t[qq], tsl)
                    else:
                        emit_prev_rope(prev, 2)
                    for kc in range(KC):
                        for j in range(4):
                            jsl = slice(j * P, (j + 1) * P)
                            nc.tensor.matmul(pv[:, jsl], xq_slj(xq, kc, jsl),
                                             wkv_sl(kc, D, 2 * D),
                                             start=(kc == 0 and j == 0),
                                             stop=(kc == KC - 1 and j == 3))
                    if qq == QT - 1:
                        emit_prev_rope(prev, 2)
                        rope(pq0, qT_t[0][qq], tsl)
                    else:
                        emit_prev_rope(prev, 3)
                    for kc in range(KC):
                        nc.tensor.matmul(pq1, wq_sl(kc, D, 2 * D),
                                         xq_sl(xq, kc),
                                         start=(kc == 0), stop=(kc == KC - 1))
                    if qq == QT - 1:
                        emit_prev_rope(prev, 3)
                        nc.scalar.copy(v_t[qq], pv)
                        rope(pq1, qT_t[1][qq], tsl)
                    prev = (pq0, pq1, pk, pv, tsl, qq)

            # ---------------- phase A: attention + chunked o-proj -----------
            with ExitStack() as actx:
                ps_pool = actx.enter_context(
                    tc.tile_pool(name="ps_pool", bufs=2, space="PSUM"))
                pv_pool = actx.enter_context(
                    tc.tile_pool(name="pv_pool", bufs=2, space="PSUM"))
                sums_pool = actx.enter_context(
                    tc.tile_pool(name="sums_pool", bufs=1, space="PSUM"))
                po_pool = actx.enter_context(
                    tc.tile_pool(name="po_pool", bufs=1, space="PSUM"))
                epool = actx.enter_context(tc.tile_pool(name="epool", bufs=4))
                elpool = actx.enter_context(tc.tile_pool(name="elpool", bufs=3))
                spool = actx.enter_context(tc.tile_pool(name="spool", bufs=2))
                bpool = actx.enter_context(tc.tile_pool(name="bpool", bufs=2))
                mpool = actx.enter_context(tc.tile_pool(name="mpool", bufs=2))
                opool = actx.enter_context(tc.tile_pool(name="opool", bufs=4))

                # build the flat chunk-job list: g outer, h inner
                class Grp:
                    pass

                jobs = []
                for g in range(G):
                    for h in range(H_LOC):
                        nkb = 2 * g + 2
                        grp = Grp()
                        grp.g, grp.h, grp.nkb = g, h, nkb
                        grp.kba = max(2 * g - 1, 0)
                        grp.kbb = grp.kba + 1
                        grp.e_map = {}
                        grp.pvacc = None
                        grp.sums = None
                        chunks = []
                        i = 0
                        while nkb - i >= 4:
                            chunks.append(list(range(i, i + 4)))
                            i += 4
                        if i < nkb:
                            chunks.append(list(range(i, i + 2)))
                        for ci, chunk in enumerate(chunks):
                            jobs.append((grp, chunk, ci == 0,
                                         ci == len(chunks) - 1))

                oproj_q = []
                copy_engines = [nc.scalar.copy, nc.vector.tensor_copy]
                copy_i = [0]

                def emit_oproj(tb, cgi, from_ps=False):
                    csl = slice(cgi * 512, (cgi + 1) * 512)
                    if from_ps:
                        # epilogue: score-chunk PSUM banks are dead, rotate po
                        # through them so chains overlap their free-up copies
                        po = ps_pool.tile([P, 1024], F32, tag="ps",
                                          name="ps")[:, 0:512]
                    else:
                        po = po_pool.tile([P, 512], F32, tag="po", name="po")
                    for hh in range(H_LOC):
                        nc.tensor.matmul(po, outT_sl(hh, tb),
                                         wo_sb[:, hh, csl],
                                         start=(hh == 0),
                                         stop=(hh == H_LOC - 1))
                    o_t = opool.tile([P, 512], BF16, tag="o_t", name="o_t")
                    copy_engines[copy_i[0] % 2](o_t, po)
                    copy_i[0] += 1
                    nc.sync.dma_start(
                        out=out[tb * P:(tb + 1) * P, csl], in_=o_t)

                def drain_oproj(n, from_ps=False):
                    for i in range(min(n, len(oproj_q))):
                        emit_oproj(*oproj_q.pop(0), from_ps=from_ps)

                def emit_scores(job):
                    grp, chunk, first, last = job
                    g, h = grp.g, grp.h
                    nkb = grp.nkb
                    if first:
                        grp.pvacc = pv_pool.tile([P, 512], F32, tag="pvacc",
                                                 name="pvacc")
                        grp.sums = sums_pool.tile([1, 512], F32, tag="sums",
                                                  name="sums")
                    ps = ps_pool.tile([P, 1024], F32, tag="ps", name="ps")
                    # the group's final key block (kb = 2g+1) is fully masked
                    # for the first 128 queries: compute only its valid 128
                    # columns. everything else is 256 wide.
                    off = 0
                    for i, kb in enumerate(chunk):
                        wkb = 128 if kb == nkb - 1 else 256
                        qlo = g * 256 + (256 - wkb)
                        nc.tensor.matmul(
                            ps[:, off:off + wkb], kT_sl(kb),
                            qT_t[h][g // 2][:, (g % 2) * 256 + (256 - wkb):
                                            (g % 2 + 1) * 256],
                            start=(i % 2 == 0), stop=(i % 2 == 1))
                        grp.e_map[kb] = (None, off, wkb)
                        off += wkb
                    w = off
                    if last:
                        nc.vector.tensor_add(ps[:, w - 384:w],
                                             ps[:, w - 384:w], maskc)
                    e = epool.tile([P, 1024], BF16, tag="e", name="e")
                    nc.scalar.activation(e[:, 0:w], ps[:, 0:w], Exp)
                    for kb in chunk:
                        _, off_kb, wkb = grp.e_map[kb]
                        grp.e_map[kb] = (e, off_kb, wkb)

                def emit_consume(job):
                    grp, chunk, first, last = job
                    g, h, nkb = grp.g, grp.h, grp.nkb
                    pvacc, sums = grp.pvacc, grp.sums
                    for i, kb in enumerate(chunk):
                        e, off, wkb = grp.e_map[kb]
                        esl = e[:, off:off + wkb]
                        qo = 256 - wkb          # query offset for narrow kb
                        vr = v_sl(kb)
                        nc.tensor.matmul(pvacc[:, qo:256], vr, esl,
                                         start=(kb == 0),
                                         stop=(kb == nkb - 1))
                        nc.tensor.matmul(sums[0:1, qo:256], ones, esl,
                                         start=(kb == 0),
                                         stop=(kb == nkb - 1))
                        if kb in (grp.kba, grp.kbb):
                            # pol/psl share the pog/psg PSUM banks: they ride
                            # on the pending-zero set by pog/psg's start=True
                            # (start=False write-then-accumulate semantics)
                            wi = 0 if kb == grp.kba else 1
                            el = elpool.tile([P, 256], BF16, tag="el",
                                             name="el")
                            nc.vector.tensor_mul(
                                el[:, 0:wkb], esl,
                                mulc[:, wi * 256 + qo:(wi + 1) * 256])
                            nc.tensor.matmul(pvacc[:, 256 + qo:512], vr,
                                             el[:, 0:wkb],
                                             start=False, stop=False,
                                             skip_group_check=True)
                            nc.tensor.matmul(sums[0:1, 256 + qo:512], ones,
                                             el[:, 0:wkb],
                                             start=False, stop=False,
                                             skip_group_check=True)
                    if last:
                        qsl = slice(g * 256, (g + 1) * 256)
                        rec = spool.tile([1, 512], F32, tag="rec", name="rec")
                        nc.vector.reciprocal(rec, sums)
                        bc = bpool.tile([P, 512], F32, tag="bc", name="bc")
                        nc.gpsimd.partition_broadcast(bc, rec)
                        m1 = mpool.tile([P, 512], F32, tag="m1", name="m1")
                        nc.vector.tensor_mul(m1, pvacc, bc)
                        nc.vector.tensor_add(outT_t[h][g],
                                             m1[:, 0:256], m1[:, 256:512])
                        if h == H_LOC - 1:
                            for tb in (2 * g, 2 * g + 1):
                                for cgi in range(4):
                                    oproj_q.append((tb, cgi))

                emit_scores(jobs[0])
                for j in range(1, len(jobs)):
                    emit_scores(jobs[j])
                    # split the o-proj drain around the consume stage so two
                    # po chains never sit back-to-back on the in-order PE
                    # (the second would stall on the first's PSUM-free copy)
                    drain_oproj(1)
                    emit_consume(jobs[j - 1])
                    drain_oproj(1, from_ps=True)
                emit_consume(jobs[-1])
                while oproj_q:
                    drain_oproj(1)
                    drain_oproj(1, from_ps=True)
    return nc


def _build_program():
    if "nc" not in _PROGRAM_CACHE:
        nc = bacc.Bacc("TRN2", target_bir_lowering=False, debug=False,
                       num_devices=N_CORES)
        _emit(nc)
        nc.compile()
        _PROGRAM_CACHE["nc"] = nc
    return _PROGRAM_CACHE["nc"]


def _feed_layout(w):
    """[C, n] weight -> [P, KC * n] with rows = SBUF partitions."""
    n = w.shape[1]
    return np.ascontiguousarray(
        w.reshape(KC, P, n).transpose(1, 0, 2).reshape(P, KC * n))


def _in_maps(x, wq, wk, wv, wo):
    x = np.asarray(x, np.float32).reshape(T, C)
    xT = x.T.astype(BF)
    # xf[q*P + p, kc*512 + t] = xT[kc*128 + p, q*512 + t]
    xf = np.ascontiguousarray(
        xT.reshape(KC, P, QT, 512).transpose(2, 1, 0, 3).reshape(
            QT * P, KC * 512))
    wq = np.asarray(wq, np.float32)
    wk = np.asarray(wk, np.float32)
    wv = np.asarray(wv, np.float32)
    wo = np.asarray(wo, np.float32)
    consts = _host_constants()
    scale = 1.0 / math.sqrt(D)
    wq_s = wq * scale
    maps = []
    for c in range(N_CORES):
        h0 = H_LOC * c
        kv = h0 // (N_HEAD // N_KV_HEAD)
        m = {
            "xf": xf,
            "wqf": _feed_layout(
                wq_s[:, h0 * D:(h0 + H_LOC) * D].astype(BF)),
            "wkvf": _feed_layout(np.concatenate(
                [wk[:, kv * D:(kv + 1) * D], wv[:, kv * D:(kv + 1) * D]],
                axis=1).astype(BF)),
            "wo": np.ascontiguousarray(
                wo[h0 * D:(h0 + H_LOC) * D, :] * 0.5).astype(BF),
        }
        m.update(consts)
        maps.append(m)
    return maps


def _run(inputs, trace=False):
    nc = _build_program()
    maps = _in_maps(inputs["x"], inputs["wq"], inputs["wk"],
                    inputs["wv"], inputs["wo"])
    res = run_bass_kernel_spmd(nc, maps, list(range(N_CORES)), trace=trace)
    total = np.zeros((T, C), np.float64)
    for rm in res.results:
        total += rm["out"].astype(np.float64)
    out = total.astype(np.float32).reshape(1, T, C)
    return out, res


def kernel(x, wq, wk, wv, wo):
    out, _ = _run({"x": x, "wq": wq, "wk": wk, "wv": wv, "wo": wo})
    return out


# revision 4
# speedup vs baseline: 1.3005x; 1.2120x over previous
"""DeepSeek-style sparse attention (causal + local-window softmax mix) on 8 trn2 cores.

v2: bf16 datapath + host-side x transpose + software-pipelined emission.

Sharding: tensor-parallel over heads. 16 Q heads / 4 KV heads; core c gets
Q heads {2c, 2c+1} and their shared KV head c//2. Each core computes a
partial output projection (its 256 rows of wo); the host sums the 8 partials.

Differences from v1:
- x is transposed and cast to bf16 on the HOST: no PE transposes, no
  PSUM->SBUF staging copies, half the x DMA traffic.
- All matmuls run in bf16 (same PE rate as fp32r at >=256-wide rhs, but
  1.0 cycles/row at any width, half the SBUF/DMA footprint).
- v is produced directly in [t, d] layout by using x^T chunks as the
  stationary operand (no v transposes).
- exp() runs on [128, 512/1024] PSUM spans (kb pairs/quads) to amortize
  the ~400-cycle Activation-engine access bubble.
- Attention emission is software-pipelined: score matmuls for chunk j+1
  are emitted before the PV/sum matmuls of chunk j, so the in-order PE
  never stalls on the Act-engine exp.
- Causal and local softmax share exp(S) tiles; sums via ones-matmuls into
  one [1,512] PSUM tile (causal|local), one reciprocal + one partition
  broadcast + a 512-wide multiply per group.
- Output partials are written bf16 (host sums in float64).
"""

import math

import numpy as np
import ml_dtypes

import concourse.bass as bass
import concourse.mybir as mybir
import concourse.tile as tile
from concourse import bacc
from concourse.bass_utils import run_bass_kernel_spmd

P = 128
T = 2048
C = 2048
D = 128                 # head dim
N_HEAD = 16
N_KV_HEAD = 4
H_LOC = 2               # q heads per core
N_CORES = 8
TB = T // P             # 16 t blocks of 128
KC = C // P             # 16 contraction chunks of 128
QT = T // 512           # 4 t quarters (projection phase)
G = T // 256            # 8 query groups of 256 (attention phase)
NEG = -1.0e30
F32 = mybir.dt.float32
BF16 = mybir.dt.bfloat16
BF = ml_dtypes.bfloat16

_PROGRAM_CACHE = {}


def _host_constants():
    """Host-precomputed constant tensors shipped as kernel inputs."""
    i = np.arange(P)
    tril = (i[:, None] <= i[None, :])          # [k_loc, q_loc]: valid iff k <= q
    mc_add = np.where(tril, 0.0, NEG).astype(np.float32)
    mc_mul = tril.astype(np.float32)
    ma_add = np.concatenate([mc_add, np.zeros((P, P), np.float32)], axis=1)
    mb_add = np.concatenate([np.full((P, P), NEG, np.float32), mc_add], axis=1)
    ma_mul = np.concatenate([mc_mul, np.ones((P, P), np.float32)], axis=1)
    mb_mul = np.concatenate([np.zeros((P, P), np.float32), mc_mul], axis=1)

    inv_freq = 1.0 / (10000.0 ** (np.arange(0, D, 2, dtype=np.float64) / D))
    t = np.arange(T, dtype=np.float64)
    freqs = t[:, None] * inv_freq[None, :]              # [T, D/2]
    emb = np.concatenate([freqs, freqs], axis=-1)       # [T, D]
    cos_t = np.cos(emb).T.astype(np.float32).copy()     # [D, T]
    sin_t = np.sin(emb).T.astype(np.float32).copy()

    # rot matmul weights: lhsT[d, d'] with out[d'] = -q[d'+64] (d'<64), q[d'-64] (d'>=64)
    rot_t = np.zeros((P, P), np.float32)
    rot_t[64 + np.arange(64), np.arange(64)] = -1.0
    rot_t[np.arange(64), 64 + np.arange(64)] = 1.0

    return {
        "mask_add": np.concatenate([ma_add, mc_add], axis=1),     # [P, 384] f32
        "mul_cat": np.concatenate([ma_mul, mb_mul], axis=1).astype(BF),
        "cos_b": cos_t.astype(BF), "sin_t": sin_t,
        "rot_t": rot_t.astype(BF),
        "ones_c": np.ones((P, 1), BF),
    }


def _emit(nc):
    # xf/wqf/wkvf are host-relaid-out so every DMA is long contiguous runs:
    # xf rows are SBUF partitions, cols are (kc, t) for one t-quarter
    xf = nc.dram_tensor("xf", [QT * P, KC * 512], BF16,
                        kind="ExternalInput").ap()
    wqf = nc.dram_tensor("wqf", [P, KC * H_LOC * D], BF16,
                         kind="ExternalInput").ap()
    wkvf = nc.dram_tensor("wkvf", [P, KC * 2 * D], BF16,
                          kind="ExternalInput").ap()
    wo = nc.dram_tensor("wo", [H_LOC * D, C], BF16, kind="ExternalInput").ap()
    cos_b_d = nc.dram_tensor("cos_b", [P, T], BF16, kind="ExternalInput").ap()
    sin_d = nc.dram_tensor("sin_t", [P, T], F32, kind="ExternalInput").ap()
    rot_d = nc.dram_tensor("rot_t", [P, P], BF16, kind="ExternalInput").ap()
    ones_d = nc.dram_tensor("ones_c", [P, 1], BF16, kind="ExternalInput").ap()
    mask_d = nc.dram_tensor("mask_add", [P, 384], F32, kind="ExternalInput").ap()
    mul_d = nc.dram_tensor("mul_cat", [P, 512], BF16, kind="ExternalInput").ap()
    out = nc.dram_tensor("out", [T, C], BF16, kind="ExternalOutput").ap()

    Exp = mybir.ActivationFunctionType.Exp

    with tile.TileContext(nc) as tc:
        from contextlib import ExitStack
        with ExitStack() as ctx:
            # DMA queue plan: projection weights FIRST on the scalar queue
            # (they gate the first PE matmul), small consts after; cos/sin
            # (needed ~10us in) then wo (needed ~halfway) on the vector
            # queue; x chunks + out stores on the sync queue.
            consts = ctx.enter_context(tc.tile_pool(name="consts", bufs=1))
            # one TILE per 4-kc chunk: tile-granular dependency tracking
            # means a single big tile would gate the first matmul on the
            # last chunk's DMA
            wq_t = []
            wkv_t = []
            kc2t = {}
            bounds = [(0, 1), (1, 5), (5, 9), (9, 13), (13, 16)]
            for ti, (lo_kc, hi_kc) in enumerate(bounds):
                n = hi_kc - lo_kc
                wqc = consts.tile([P, n, H_LOC * D], BF16, name=f"wq{ti}")
                nc.scalar.dma_start(
                    out=wqc, in_=wqf[:, lo_kc * 256:hi_kc * 256])
                wq_t.append(wqc)
                wkvc = consts.tile([P, n, 2 * D], BF16, name=f"wkv{ti}")
                nc.scalar.dma_start(
                    out=wkvc, in_=wkvf[:, lo_kc * 256:hi_kc * 256])
                wkv_t.append(wkvc)
                for kc in range(lo_kc, hi_kc):
                    kc2t[kc] = (ti, kc - lo_kc)

            def wq_sl(kc, lo, hi):
                ti, off = kc2t[kc]
                return wq_t[ti][:, off, lo:hi]

            def wkv_sl(kc, lo, hi):
                ti, off = kc2t[kc]
                return wkv_t[ti][:, off, lo:hi]
            rot_sb = consts.tile([P, P], BF16)
            nc.scalar.dma_start(out=rot_sb, in_=rot_d)
            ones = consts.tile([P, 1], BF16)
            nc.scalar.dma_start(out=ones, in_=ones_d)
            maskc = consts.tile([P, 384], F32)
            nc.scalar.dma_start(out=maskc, in_=mask_d)
            mulc = consts.tile([P, 512], BF16)
            nc.scalar.dma_start(out=mulc, in_=mul_d)
            cos_sb = consts.tile([P, T], BF16)
            nc.scalar.dma_start(out=cos_sb, in_=cos_b_d)
            sin_sb = consts.tile([P, T], F32)
            nc.scalar.dma_start(out=sin_sb, in_=sin_d)

            # persistent activations, split per quarter/group so readers
            # depend only on the slice they touch (tile-granular deps)
            persist = ctx.enter_context(tc.tile_pool(name="persist", bufs=1))
            qT_t = [[persist.tile([P, 512], BF16, name=f"qT{h}_{q}")
                     for q in range(QT)] for h in range(H_LOC)]
            kT_t = [persist.tile([P, 512], BF16, name=f"kT_{q}")
                    for q in range(QT)]
            v_t = [persist.tile([P, 512], BF16, name=f"v_{q}")
                   for q in range(QT)]
            wo_sb = persist.tile([P, H_LOC, C], BF16)
            nc.scalar.dma_start(
                out=wo_sb, in_=wo.rearrange("(h p) n -> p h n", p=P))
            outT_t = [[persist.tile([P, 256], BF16, name=f"oT{h}_{g}")
                       for g in range(G)] for h in range(H_LOC)]

            def kT_sl(kb):
                return kT_t[kb // 4][:, (kb % 4) * P:(kb % 4 + 1) * P]

            def v_sl(kb):
                return v_t[kb // 4][:, (kb % 4) * D:(kb % 4 + 1) * D]

            def qT_sl(h, g):
                return qT_t[h][g // 2][:, (g % 2) * 256:(g % 2 + 1) * 256]

            def outT_sl(h, tb):
                return outT_t[h][tb // 2][:, (tb % 2) * P:(tb % 2 + 1) * P]

            # ---------------- phase P: projections + rope -------------------
            with ExitStack() as pctx:
                xpool = pctx.enter_context(tc.tile_pool(name="xpool", bufs=3))
                ps_proj = pctx.enter_context(
                    tc.tile_pool(name="ps_proj", bufs=1, space="PSUM"))
                ps_rot = pctx.enter_context(
                    tc.tile_pool(name="ps_rot", bufs=2, space="PSUM"))
                raws = pctx.enter_context(tc.tile_pool(name="raws", bufs=4))
                t12 = pctx.enter_context(tc.tile_pool(name="t12", bufs=3))

                def rope(ps_raw, dst, tsl):
                    raw = raws.tile([P, 512], BF16, tag="raw", name="raw")
                    nc.scalar.copy(raw, ps_raw)
                    prot = ps_rot.tile([P, 512], F32, tag="prot", name="prot")
                    nc.tensor.matmul(prot, rot_sb, raw, start=True, stop=True)
                    t1 = t12.tile([P, 512], F32, tag="t1", name="t1")
                    nc.vector.tensor_mul(t1, prot, sin_sb[:, tsl])
                    t2 = t12.tile([P, 512], F32, tag="t2", name="t2")
                    nc.vector.tensor_mul(t2, raw, cos_sb[:, tsl])
                    nc.vector.tensor_add(dst, t1, t2)

                def load_xq(qq):
                    xq = []
                    rows = slice(qq * P, (qq + 1) * P)
                    for j4 in range(4):
                        xc = xpool.tile([P, 4, 512], BF16, tag=f"xq{j4}",
                                        name=f"xq{j4}")
                        nc.sync.dma_start(
                            out=xc, in_=xf[rows, j4 * 2048:(j4 + 1) * 2048])
                        xq.append(xc)
                    return xq

                def xq_sl(xq, kc):
                    return xq[kc // 4][:, kc % 4, :]

                def xq_slj(xq, kc, jsl):
                    return xq[kc // 4][:, kc % 4, jsl]

                def proj_tiles():
                    pq0 = ps_proj.tile([P, 512], F32, tag="pq0", name="pq0")
                    pq1 = ps_proj.tile([P, 512], F32, tag="pq1", name="pq1")
                    pk = ps_proj.tile([P, 512], F32, tag="pk", name="pk")
                    pv = ps_proj.tile([P, 512], F32, tag="pv", name="pv")
                    return pq0, pq1, pk, pv

                def emit_prev_rope(prev, i):
                    # rope/v-copy of the previous quarter's tensor i, emitted
                    # just before this quarter's chain reuses its PSUM bank
                    if prev is None:
                        return
                    pq0, pq1, pk, pv, ptsl, pqq = prev
                    if i == 0:   # kT first: phase A's first scores need it
                        rope(pk, kT_t[pqq], ptsl)
                    elif i == 1:
                        rope(pq0, qT_t[0][pqq], ptsl)
                    elif i == 2:
                        nc.scalar.copy(v_t[pqq], pv)
                    else:
                        rope(pq1, qT_t[1][pqq], ptsl)

                # quarter 0: kc-interleaved so PE consumption matches x DMA
                # arrival order (no pstate-resetting stalls at warmup)
                xq = load_xq(0)
                pq0, pq1, pk, pv = proj_tiles()
                for kc in range(KC):
                    st, sp = kc == 0, kc == KC - 1
                    nc.tensor.matmul(pq0, wq_sl(kc, 0, D), xq_sl(xq, kc),
                                     start=st, stop=sp)
                    nc.tensor.matmul(pq1, wq_sl(kc, D, 2 * D),
                                     xq_sl(xq, kc), start=st, stop=sp)
                    nc.tensor.matmul(pk, wkv_sl(kc, 0, D), xq_sl(xq, kc),
                                     start=st, stop=sp)
                    # v directly in [t, d] layout: x^T chunk stationary.
                    # all 4 j-blocks share one PSUM bank: only (kc0, j0)
                    # starts the zero region, only (kc15, j3) stops it
                    for j in range(4):
                        jsl = slice(j * P, (j + 1) * P)
                        nc.tensor.matmul(pv[:, jsl], xq_slj(xq, kc, jsl),
                                         wkv_sl(kc, D, 2 * D),
                                         start=(st and j == 0),
                                         stop=(sp and j == 3))
                prev = (pq0, pq1, pk, pv, slice(0, 512), 0)

                # quarters 1-3: per-tensor chains, interleaved with the
                # previous quarter's rope/v-copy (whose PSUM banks they reuse)
                for qq in range(1, QT):
                    tsl = slice(qq * 512, (qq + 1) * 512)
                    xq = load_xq(qq)
                    emit_prev_rope(prev, 0)
                    pq0, pq1, pk, pv = proj_tiles()
                    for kc in range(KC):
                        nc.tensor.matmul(pk, wkv_sl(kc, 0, D), xq_sl(xq, kc),
                                         start=(kc == 0), stop=(kc == KC - 1))
                    emit_prev_rope(prev, 1)
                    for kc in range(KC):
                        nc.tensor.matmul(pq0, wq_sl(kc, 0, D), xq_sl(xq, kc),
                                         start=(kc == 0), stop=(kc == KC - 1))
                    if qq == QT - 1:
                        # last quarter: rope its own kT mid-quarter so phase
                        # A's first score matmuls aren't gated on the tail
                        rope(pk, kT_t[qq], tsl)
                    else:
                        emit_prev_rope(prev, 2)
                    for kc in range(KC):
                        for j in range(4):
                            jsl = slice(j * P, (j + 1) * P)
                            nc.tensor.matmul(pv[:, jsl], xq_slj(xq, kc, jsl),
                                             wkv_sl(kc, D, 2 * D),
                                             start=(kc == 0 and j == 0),
                                             stop=(kc == KC - 1 and j == 3))
                    if qq == QT - 1:
                        emit_prev_rope(prev, 2)
                        rope(pq0, qT_t[0][qq], tsl)
                    else:
                        emit_prev_rope(prev, 3)
                    for kc in range(KC):
                        nc.tensor.matmul(pq1, wq_sl(kc, D, 2 * D),
                                         xq_sl(xq, kc),
                                         start=(kc == 0), stop=(kc == KC - 1))
                    if qq == QT - 1:
                        emit_prev_rope(prev, 3)
                        nc.scalar.copy(v_t[qq], pv)
                        rope(pq1, qT_t[1][qq], tsl)
                    prev = (pq0, pq1, pk, pv, tsl, qq)

            # ---------------- phase A: attention + chunked o-proj -----------
            with ExitStack() as actx:
                ps_pool = actx.enter_context(
                    tc.tile_pool(name="ps_pool", bufs=2, space="PSUM"))
                pv_pool = actx.enter_context(
                    tc.tile_pool(name="pv_pool", bufs=2, space="PSUM"))
                sums_pool = actx.enter_context(
                    tc.tile_pool(name="sums_pool", bufs=1, space="PSUM"))
                po_pool = actx.enter_context(
                    tc.tile_pool(name="po_pool", bufs=1, space="PSUM"))
                epool = actx.enter_context(tc.tile_pool(name="epool", bufs=6))
                elpool = actx.enter_context(tc.tile_pool(name="elpool", bufs=4))
                spool = actx.enter_context(tc.tile_pool(name="spool", bufs=3))
                bpool = actx.enter_context(tc.tile_pool(name="bpool", bufs=3))
                mpool = actx.enter_context(tc.tile_pool(name="mpool", bufs=3))
                opool = actx.enter_context(tc.tile_pool(name="opool", bufs=6))

                # build the flat chunk-job list: g outer, h inner
                class Grp:
                    pass

                jobs = []
                for g in range(G):
                    for h in range(H_LOC):
                        nkb = 2 * g + 2
                        grp = Grp()
                        grp.g, grp.h, grp.nkb = g, h, nkb
                        grp.kba = max(2 * g - 1, 0)
                        grp.kbb = grp.kba + 1
                        grp.e_map = {}
                        grp.pvacc = None
                        grp.sums = None
                        chunks = []
                        i = 0
                        while nkb - i >= 4:
                            chunks.append(list(range(i, i + 4)))
                            i += 4
                        if i < nkb:
                            chunks.append(list(range(i, i + 2)))
                        for ci, chunk in enumerate(chunks):
                            jobs.append((grp, chunk, ci == 0,
                                         ci == len(chunks) - 1))

                oproj_q = []
                copy_engines = [nc.scalar.copy, nc.vector.tensor_copy]
                copy_i = [0]

                def emit_oproj(tb, cgi, from_ps=False):
                    csl = slice(cgi * 512, (cgi + 1) * 512)
                    if from_ps:
                        # epilogue: score-chunk PSUM banks are dead, rotate po
                        # through them so chains overlap their free-up copies
                        po = ps_pool.tile([P, 1024], F32, tag="ps",
                                          name="ps")[:, 0:512]
                    else:
                        po = po_pool.tile([P, 512], F32, tag="po", name="po")
                    for hh in range(H_LOC):
                        nc.tensor.matmul(po, outT_sl(hh, tb),
                                         wo_sb[:, hh, csl],
                                         start=(hh == 0),
                                         stop=(hh == H_LOC - 1))
                    o_t = opool.tile([P, 512], BF16, tag="o_t", name="o_t")
                    copy_engines[copy_i[0] % 2](o_t, po)
                    copy_i[0] += 1
                    nc.sync.dma_start(
                        out=out[tb * P:(tb + 1) * P, csl], in_=o_t)

                def drain_oproj(n, from_ps=False):
                    for i in range(min(n, len(oproj_q))):
                        emit_oproj(*oproj_q.pop(0), from_ps=from_ps)

                def emit_scores(job):
                    grp, chunk, first, last = job
                    g, h = grp.g, grp.h
                    nkb = grp.nkb
                    if first:
                        grp.pvacc = pv_pool.tile([P, 512], F32, tag="pvacc",
                                                 name="pvacc")
                        grp.sums = sums_pool.tile([1, 512], F32, tag="sums",
                                                  name="sums")
                    ps = ps_pool.tile([P, 1024], F32, tag="ps", name="ps")
                    # the group's final key block (kb = 2g+1) is fully masked
                    # for the first 128 queries: compute only its valid 128
                    # columns. everything else is 256 wide.
                    off = 0
                    for i, kb in enumerate(chunk):
                        wkb = 128 if kb == nkb - 1 else 256
                        qlo = g * 256 + (256 - wkb)
                        nc.tensor.matmul(
                            ps[:, off:off + wkb], kT_sl(kb),
                            qT_t[h][g // 2][:, (g % 2) * 256 + (256 - wkb):
                                            (g % 2 + 1) * 256],
                            start=(i % 2 == 0), stop=(i % 2 == 1))
                        grp.e_map[kb] = (None, off, wkb)
                        off += wkb
                    w = off
                    if last:
                        nc.vector.tensor_add(ps[:, w - 384:w],
                                             ps[:, w - 384:w], maskc)
                    e = epool.tile([P, 1024], BF16, tag="e", name="e")
                    nc.scalar.activation(e[:, 0:w], ps[:, 0:w], Exp)
                    for kb in chunk:
                        _, off_kb, wkb = grp.e_map[kb]
                        grp.e_map[kb] = (e, off_kb, wkb)


                def emit_consume(job):
                    grp, chunk, first, last = job
                    g, h, nkb = grp.g, grp.h, grp.nkb
                    pvacc, sums = grp.pvacc, grp.sums
                    for i, kb in enumerate(chunk):
                        e, off, wkb = grp.e_map[kb]
                        esl = e[:, off:off + wkb]
                        qo = 256 - wkb          # query offset for narrow kb
                        vr = v_sl(kb)
                        nc.tensor.matmul(pvacc[:, qo:256], vr, esl,
                                         start=(kb == 0),
                                         stop=(kb == nkb - 1))
                        nc.tensor.matmul(sums[0:1, qo:256], ones, esl,
                                         start=(kb == 0),
                                         stop=(kb == nkb - 1))
                        if kb in (grp.kba, grp.kbb):
                            # pol/psl share the pog/psg PSUM banks: they ride
                            # on the pending-zero set by pog/psg's start=True
                            # (start=False write-then-accumulate semantics)
                            wi = 0 if kb == grp.kba else 1
                            el = elpool.tile([P, 256], BF16, tag="el",
                                             name="el")
                            nc.vector.tensor_mul(
                                el[:, 0:wkb], esl,
                                mulc[:, wi * 256 + qo:(wi + 1) * 256])
                            nc.tensor.matmul(pvacc[:, 256 + qo:512], vr,
                                             el[:, 0:wkb],
                                             start=False, stop=False,
                                             skip_group_check=True)
                            nc.tensor.matmul(sums[0:1, 256 + qo:512], ones,
                                             el[:, 0:wkb],
                                             start=False, stop=False,
                                             skip_group_check=True)
                    if last:
                        qsl = slice(g * 256, (g + 1) * 256)
                        rec = spool.tile([1, 512], F32, tag="rec", name="rec")
                        nc.vector.reciprocal(rec, sums)
                        bc = bpool.tile([P, 512], F32, tag="bc", name="bc")
                        nc.gpsimd.partition_broadcast(bc, rec)
                        m1 = mpool.tile([P, 512], F32, tag="m1", name="m1")
                        nc.vector.tensor_mul(m1, pvacc, bc)
                        nc.vector.tensor_add(outT_t[h][g],
                                             m1[:, 0:256], m1[:, 256:512])
                        if h == H_LOC - 1:
                            for tb in (2 * g, 2 * g + 1):
                                for cgi in range(4):
                                    oproj_q.append((tb, cgi))

                emit_scores(jobs[0])
                for j in range(1, len(jobs)):
                    emit_scores(jobs[j])
                    # split the o-proj drain around the consume stage so two
                    # po chains never sit back-to-back on the in-order PE
                    # (the second would stall on the first's PSUM-free copy)
                    drain_oproj(1)
                    emit_consume(jobs[j - 1])
                    drain_oproj(1, from_ps=True)
                emit_consume(jobs[-1])
                while oproj_q:
                    drain_oproj(1)
                    drain_oproj(1, from_ps=True)
    return nc


def _build_program():
    if "nc" not in _PROGRAM_CACHE:
        nc = bacc.Bacc("TRN2", target_bir_lowering=False, debug=False,
                       num_devices=N_CORES)
        _emit(nc)
        nc.compile()
        _PROGRAM_CACHE["nc"] = nc
    return _PROGRAM_CACHE["nc"]


def _feed_layout(w):
    """[C, n] weight -> [P, KC * n] with rows = SBUF partitions."""
    n = w.shape[1]
    return np.ascontiguousarray(
        w.reshape(KC, P, n).transpose(1, 0, 2).reshape(P, KC * n))


def _in_maps(x, wq, wk, wv, wo):
    x = np.asarray(x, np.float32).reshape(T, C)
    xT = x.T.astype(BF)
    # xf[q*P + p, kc*512 + t] = xT[kc*128 + p, q*512 + t]
    xf = np.ascontiguousarray(
        xT.reshape(KC, P, QT, 512).transpose(2, 1, 0, 3).reshape(
            QT * P, KC * 512))
    wq = np.asarray(wq, np.float32)
    wk = np.asarray(wk, np.float32)
    wv = np.asarray(wv, np.float32)
    wo = np.asarray(wo, np.float32)
    consts = _host_constants()
    scale = 1.0 / math.sqrt(D)
    wq_s = wq * scale
    maps = []
    for c in range(N_CORES):
        h0 = H_LOC * c
        kv = h0 // (N_HEAD // N_KV_HEAD)
        m = {
            "xf": xf,
            "wqf": _feed_layout(
                wq_s[:, h0 * D:(h0 + H_LOC) * D].astype(BF)),
            "wkvf": _feed_layout(np.concatenate(
                [wk[:, kv * D:(kv + 1) * D], wv[:, kv * D:(kv + 1) * D]],
                axis=1).astype(BF)),
            "wo": np.ascontiguousarray(
                wo[h0 * D:(h0 + H_LOC) * D, :] * 0.5).astype(BF),
        }
        m.update(consts)
        maps.append(m)
    return maps


def _run(inputs, trace=False):
    nc = _build_program()
    maps = _in_maps(inputs["x"], inputs["wq"], inputs["wk"],
                    inputs["wv"], inputs["wo"])
    res = run_bass_kernel_spmd(nc, maps, list(range(N_CORES)), trace=trace)
    total = np.zeros((T, C), np.float64)
    for rm in res.results:
        total += rm["out"].astype(np.float64)
    out = total.astype(np.float32).reshape(1, T, C)
    return out, res


def kernel(x, wq, wk, wv, wo):
    out, _ = _run({"x": x, "wq": wq, "wk": wk, "wv": wv, "wo": wo})
    return out
